# revision 12
# baseline (speedup 1.0000x reference)
"""Trainium2 Bass kernel for DynamicPTTopicModeling.

Computes, per batch b (one batch per NeuronCore, 8 cores):
    qg  = relu(qz @ bw.T)            # [R=8192, G=512], contraction over d=1024
    den = max(sum_g qg, 1e-6)        # per-row L1 norm
    msg = (qg @ bw) / den            # [R, D=1024]

Sharding: batch b across the 8 NeuronCores, fully data-parallel (one batch
per core, no collectives).

The PE contracts over the partition dim for both operands, so qz must enter
mm1 with d on partitions. kernel() transposes qz/bw on the host while
building the per-core shards (layout marshalling, same class as the
reshape/shard step) so the device runs a pure matmul stream.

All tensors move and compute in bf16 (PSUM accumulation stays fp32):
  - same PE throughput as f32r (1 cycle/row), but weight loads get FWL
    (fast weight load) instead of the ~427-cycle fp32 path, so LDWEIGHTS
    fully hides under the 512-cycle matmuls;
  - halves DMA bytes in both directions, which removes the PE starvation
    in the first ~3 mega-tiles and halves the output tail;
  - measured end-to-end relative error ~3e-3 vs the fp32 reference
    (tolerance 2e-2): the matmul chain is short and PSUM accumulates fp32.

Per-core strategy (16 "mega-tiles" of 512 rows):
  - mm1 produces qg TRANSPOSED ([g, p] layout): stationary = host-pretransposed
    bwT slices, moving = host-pretransposed qzT chunks (N=512). mm2 then
    consumes qg slices directly as its stationary with bw natural as moving.
  - Row-sums over g (the partition dim here) ride mm2: after each 128-row
    block's first mm2 group, four 1-column matmuls qgr_slice.T @ ones
    accumulate sum_g qg into a [128, 1] PSUM column — same stationary and
    same dependencies as the mm2 matmuls (so they never add a wait), and
    den lands directly in per-partition layout for the scaled drain.
    No ones-stationary row-sum passes, no PE transposes.

Schedule notes:
  - The PE clock is gated by HAM: cold = 1.2 GHz until ~3.4us of sustained
    activity. A burst of dummy warm-up matmuls runs while the first DMAs are
    in flight so the real stream starts at 2.4 GHz.
  - Trn2 has two HWDGE rings (sync + ACT), FIFO per issuing engine. Sync
    ring = qzT loads only; ACT ring = weights then msg stores. All scaled
    PSUM->SBUF multiplies run on DVE so the ACT queue is just relu +
    dma_start and a waiting store never delays compute or loads.
"""
from contextlib import ExitStack

import ml_dtypes
import numpy as np

import concourse.bass as bass
import concourse.tile as tile
from concourse import bacc, mybir
from concourse.bass_utils import run_bass_kernel_spmd

F32 = mybir.dt.float32
BF16 = mybir.dt.bfloat16
AF = mybir.ActivationFunctionType
NP_BF16 = ml_dtypes.bfloat16

B, C, P, D, G = 8, 16, 512, 1024, 512
R = C * P            # 8192 rows per batch
MEGA = 512           # rows per mega-tile
NSUB = MEGA // 128   # 4
NMEGA = R // MEGA    # 16
KD = D // 128        # 8 d-chunks
KG = G // 128        # 4 g-chunks
EPS = 1e-6
N_CORES = 8
N_WARM = 6           # dummy matmuls to flip the HAM clock gate while DMAs fly


def build_kernel():
    nc = bacc.Bacc("TRN2", target_bir_lowering=False)
    # Inputs are host-pretransposed and host-converted to bf16.
    qzT_d = nc.dram_tensor("qzT", [D, R], BF16, kind="ExternalInput")
    bw_d = nc.dram_tensor("bw", [G, D], BF16, kind="ExternalInput")
    bwT_d = nc.dram_tensor("bwT", [D, G], BF16, kind="ExternalInput")
    msg_d = nc.dram_tensor("msg", [R, D], BF16, kind="ExternalOutput")

    with tile.TileContext(nc) as tc, ExitStack() as ctx:
        const_pool = ctx.enter_context(tc.tile_pool(name="const", bufs=1))
        in_pool = ctx.enter_context(tc.tile_pool(name="inp", bufs=3))
        qgr_pool = ctx.enter_context(tc.tile_pool(name="qgrp", bufs=2))
        out_pool = ctx.enter_context(tc.tile_pool(name="outp", bufs=2))
        small_pool = ctx.enter_context(tc.tile_pool(name="smallp", bufs=2))
        qg_psum = ctx.enter_context(tc.tile_pool(name="qgps", bufs=4, space="PSUM"))
        msg_psum = ctx.enter_context(tc.tile_pool(name="msgps", bufs=4, space="PSUM"))
        # warm/rowsum psum tiles share the qg pool's slots (tag "qg_ps")
        rs_psum = qg_psum

        # Weights go on the second HWDGE ring (nc.scalar) so they don't queue
        # behind the qzT stream; bwT first and in quarters — it gates mm1.
        # bwT [d, g] -> [128, k, G]  (mm1 stationary)
        bwT_sb = const_pool.tile([128, KD, G], BF16)
        bwT_view = bwT_d[:].rearrange("(k p) g -> p k g", p=128)
        for q in range(2):
            nc.scalar.dma_start(
                out=bwT_sb[:, 4 * q:4 * q + 4, :], in_=bwT_view[:, 4 * q:4 * q + 4, :]
            )
        # bw natural [g, d] -> [128, gc, d]  (mm2 moving operand); its DMAs
        # are emitted after the first two qzT loads — mm2 doesn't need it
        # until ~15us, while mm1 starves without early qzT
        bw_sb = const_pool.tile([128, KG, D], BF16)
        bw_view = bw_d[:].rearrange("(gc p) d -> p gc d", p=128)

        def load_bw():
            for q in range(4):
                nc.scalar.dma_start(
                    out=bw_sb[:, q:q + 1, :], in_=bw_view[:, q:q + 1, :]
                )

        # ones column: moving operand of the row-sum matmuls (and warm-up
        # stationary)
        ones_c = const_pool.tile([128, 1], BF16)
        nc.vector.memset(ones_c, 1.0)

        # HAM warm-up: the PE boots throttled to 1.2 GHz and only reaches
        # 2.4 GHz after ~3.4us of sustained activity. Burn that window on
        # dummy matmuls while the first qzT/bwT chunks are still in flight.
        warm_mv = const_pool.tile([128, MEGA], BF16)
        nc.vector.memset(warm_mv, 0.0)
        warm_ps = qg_psum.tile([1, MEGA], F32, name="warm_ps", tag="qg_ps")
        for w in range(N_WARM):
            nc.tensor.matmul(
                warm_ps, ones_c, warm_mv,
                start=(w == 0), stop=(w == N_WARM - 1),
            )

        def load_qzT(t):
            # The first two megas are wanted faster than one HWDGE ring can
            # deliver (the PE eats 1MB in ~7us but also mm2 needs bw soon):
            # split their chunks across BOTH rings so the aggregate ~358GB/s
            # HBM rate applies. Steady state: 512KB halves on the sync ring.
            qzT = in_pool.tile([128, KD, MEGA], BF16, name="qzT")
            qzT_view = qzT_d[:, t * MEGA:(t + 1) * MEGA].rearrange(
                "(k p) r -> p k r", p=128
            )
            nq = 4 if t < 3 else 2
            step = KD // nq
            for q in range(nq):
                eng = nc.scalar if (t < 2 and q >= nq // 2) else nc.sync
                eng.dma_start(
                    out=qzT[:, step * q:step * (q + 1), :],
                    in_=qzT_view[:, step * q:step * (q + 1), :],
                )
            return qzT

        # Load issues are software-pipelined two megas ahead; the sync ring
        # carries nothing but these loads so they are never head-of-line
        # blocked by a store's semaphore wait.
        pend_qzT = [load_qzT(0), load_qzT(1)]
        load_bw()

        for t in range(NMEGA):
            qzT = pend_qzT.pop(0)
            if t + 2 < NMEGA:
                pend_qzT.append(load_qzT(t + 2))

            # ---- mm1: qgT[gc] = sum_k bwT[:,k,gc].T @ qzT[:,k,:]  -> relu ----
            qgr = qgr_pool.tile([128, KG, MEGA], BF16, name="qgr")
            rsc_ps = None
            for gc in range(KG):
                qg_ps = qg_psum.tile([128, MEGA], F32, name="qg_ps")
                for k in range(KD):
                    nc.tensor.matmul(
                        qg_ps,
                        bwT_sb[:, k, gc * 128:(gc + 1) * 128],
                        qzT[:, k, :],
                        start=(k == 0),
                        stop=(k == KD - 1),
                    )
                if gc < KG - 1:
                    nc.scalar.activation(qgr[:, gc, :], qg_ps, AF.Relu)
                else:
                    # last chunk's relu in 128-col blocks: mm2's gc3
                    # accumulation (and the s0 rowsum) start ~650ns after
                    # mm1 ends — a whole-tile relu (~430ns) occasionally
                    # loses that race; per-block it never does
                    for ss in range(NSUB):
                        nc.scalar.activation(
                            qgr[:, gc, ss * 128:(ss + 1) * 128],
                            qg_ps[:, ss * 128:(ss + 1) * 128],
                            AF.Relu,
                        )
                if gc == 1:
                    # allocated after gc0/gc1's psum tiles so the pool
                    # rotation never makes an mm1 group wait on the
                    # still-live rowsum column tile
                    rsc_ps = rs_psum.tile([128, NSUB], F32, name="rsc_ps", tag="qg_ps")

            # ---- mm2: msg[s] = sum_gc qgr[:,gc,s].T @ bw[gc], scaled ----
            # Row-sum columns ride between the two halves: for each s, four
            # 1-column matmuls (same stationary slices as mm2, so their relu
            # dependencies are already satisfied) put sum_g qg[g, p] into
            # rsc_ps[:, s]; max+reciprocal on DVE overlap the h=1 half.
            msg_sb = out_pool.tile([128, NSUB, D], BF16, name="msg_sb")
            sc_sb = small_pool.tile([128, NSUB], F32, name="sc_sb")
            for s in range(NSUB):
                m_ps = []
                for h in range(2):
                    mp = msg_psum.tile([128, 512], F32, name="m_ps")
                    m_ps.append(mp)
                    for gc in range(KG):
                        nc.tensor.matmul(
                            mp,
                            qgr[:, gc, s * 128:(s + 1) * 128],
                            bw_sb[:, gc, h * 512:(h + 1) * 512],
                            start=(gc == 0),
                            stop=(gc == KG - 1),
                        )
                    if h == 0:
                        for gc in range(KG):
                            nc.tensor.matmul(
                                rsc_ps[:, s:s + 1],
                                qgr[:, gc, s * 128:(s + 1) * 128],
                                ones_c,
                                start=(gc == 0),
                                stop=(gc == KG - 1),
                                skip_group_check=True,
                            )
                        nc.vector.tensor_scalar_max(
                            sc_sb[:, s:s + 1], rsc_ps[:, s:s + 1], EPS
                        )
                        nc.vector.reciprocal(sc_sb[:, s:s + 1], sc_sb[:, s:s + 1])
                for h in range(2):
                    nc.vector.tensor_scalar_mul(
                        msg_sb[:, s, h * 512:(h + 1) * 512],
                        m_ps[h],
                        sc_sb[:, s:s + 1],
                    )
                # output stores ride the ACT ring (weights are long done);
                # the sync ring stays loads-only
                nc.scalar.dma_start(
                    out=msg_d[t * MEGA + s * 128:t * MEGA + (s + 1) * 128, :],
                    in_=msg_sb[:, s, :],
                )

    nc.compile()
    return nc


_NC_CACHE = None


def _get_nc():
    global _NC_CACHE
    if _NC_CACHE is None:
        _NC_CACHE = build_kernel()
    return _NC_CACHE


def kernel(qz: np.ndarray, binary_weight: np.ndarray) -> np.ndarray:
    qz = np.asarray(qz, dtype=np.float32)
    bw = np.ascontiguousarray(np.asarray(binary_weight, dtype=np.float32))
    assert qz.shape == (B, C, P, D), qz.shape
    assert bw.shape == (B, G, D), bw.shape

    nc = _get_nc()
    in_maps = []
    for i in range(N_CORES):
        qzT = np.ascontiguousarray(qz[i].reshape(R, D).T).astype(NP_BF16)
        bwi = bw[i].astype(NP_BF16)                              # [G, D]
        bwT = np.ascontiguousarray(bw[i].T).astype(NP_BF16)      # [D, G]
        in_maps.append({"qzT": qzT, "bw": bwi, "bwT": bwT})
    res = run_bass_kernel_spmd(nc, in_maps, core_ids=list(range(N_CORES)))
    out = np.stack(
        [
            res.results[i]["msg"].astype(np.float32).reshape(C, P, D)
            for i in range(N_CORES)
        ],
        axis=0,
    )
    return out


# revision 13
# speedup vs baseline: 1.1845x; 1.1845x over previous
"""Trainium2 Bass kernel for DynamicPTTopicModeling.

Computes, per batch b (one batch per NeuronCore, 8 cores):
    qg  = relu(qz @ bw.T)            # [R=8192, G=512], contraction over d=1024
    den = max(sum_g qg, 1e-6)        # per-row L1 norm
    msg = (qg @ bw) / den            # [R, D=1024]

Sharding: batch b across the 8 NeuronCores, fully data-parallel (one batch
per core, no collectives).

The PE contracts over the partition dim for both operands, so qz must enter
mm1 with d on partitions. kernel() transposes qz/bw on the host while
building the per-core shards (layout marshalling, same class as the
reshape/shard step) so the device runs a pure matmul stream.

All tensors move and compute in bf16 (PSUM accumulation stays fp32):
  - same PE throughput as f32r (1 cycle/row), but weight loads get FWL
    (fast weight load) instead of the ~427-cycle fp32 path, so LDWEIGHTS
    fully hides under the 512-cycle matmuls;
  - halves DMA bytes in both directions, which removes the PE starvation
    in the first ~3 mega-tiles and halves the output tail;
  - measured end-to-end relative error ~3e-3 vs the fp32 reference
    (tolerance 2e-2): the matmul chain is short and PSUM accumulates fp32.

Per-core strategy (16 "mega-tiles" of 512 rows):
  - mm1 produces qg TRANSPOSED ([g, p] layout): stationary = host-pretransposed
    bwT slices, moving = host-pretransposed qzT chunks (N=512). mm2 then
    consumes qg slices directly as its stationary with bw natural as moving.
  - Row-sums over g (the partition dim here) ride mm2: after each 128-row
    block's first mm2 group, four 1-column matmuls qgr_slice.T @ ones
    accumulate sum_g qg into a [128, 1] PSUM column — same stationary and
    same dependencies as the mm2 matmuls (so they never add a wait), and
    den lands directly in per-partition layout for the scaled drain.
    No ones-stationary row-sum passes, no PE transposes.

Schedule notes:
  - The PE clock is gated by HAM: cold = 1.2 GHz until ~3.4us of sustained
    activity. A burst of dummy warm-up matmuls runs while the first DMAs are
    in flight so the real stream starts at 2.4 GHz.
  - Trn2 has two HWDGE rings (sync + ACT), FIFO per issuing engine. Sync
    ring = qzT loads only; ACT ring = weights then msg stores. All scaled
    PSUM->SBUF multiplies run on DVE so the ACT queue is just relu +
    dma_start and a waiting store never delays compute or loads.
"""
from contextlib import ExitStack

import ml_dtypes
import numpy as np

import concourse.bass as bass
import concourse.tile as tile
from concourse import bacc, mybir
from concourse.bass_utils import run_bass_kernel_spmd

F32 = mybir.dt.float32
BF16 = mybir.dt.bfloat16
AF = mybir.ActivationFunctionType
NP_BF16 = ml_dtypes.bfloat16

B, C, P, D, G = 8, 16, 512, 1024, 512
R = C * P            # 8192 rows per batch
MEGA = 512           # rows per mega-tile
NSUB = MEGA // 128   # 4
NMEGA = R // MEGA    # 16
KD = D // 128        # 8 d-chunks
KG = G // 128        # 4 g-chunks
EPS = 1e-6
N_CORES = 8
N_WARM = 10          # dummy matmuls to flip the HAM clock gate while DMAs fly


def build_kernel():
    nc = bacc.Bacc("TRN2", target_bir_lowering=False)
    # Inputs are host-pretransposed and host-converted to bf16.
    qzT_d = nc.dram_tensor("qzT", [D, R], BF16, kind="ExternalInput")
    bw_d = nc.dram_tensor("bw", [G, D], BF16, kind="ExternalInput")
    bwT_d = nc.dram_tensor("bwT", [D, G], BF16, kind="ExternalInput")
    msg_d = nc.dram_tensor("msg", [R, D], BF16, kind="ExternalOutput")

    with tile.TileContext(nc) as tc, ExitStack() as ctx:
        const_pool = ctx.enter_context(tc.tile_pool(name="const", bufs=1))
        in_pool = ctx.enter_context(tc.tile_pool(name="inp", bufs=3))
        qgr_pool = ctx.enter_context(tc.tile_pool(name="qgrp", bufs=2))
        out_pool = ctx.enter_context(tc.tile_pool(name="outp", bufs=2))
        small_pool = ctx.enter_context(tc.tile_pool(name="smallp", bufs=2))
        qg_psum = ctx.enter_context(tc.tile_pool(name="qgps", bufs=4, space="PSUM"))
        msg_psum = ctx.enter_context(tc.tile_pool(name="msgps", bufs=4, space="PSUM"))
        # warm/rowsum psum tiles share the qg pool's slots (tag "qg_ps")
        rs_psum = qg_psum

        # Weights go on the second HWDGE ring (nc.scalar) so they don't queue
        # behind the qzT stream; bwT first and in quarters — it gates mm1.
        # bwT [d, g] -> [128, k, G]  (mm1 stationary)
        bwT_sb = const_pool.tile([128, KD, G], BF16)
        bwT_view = bwT_d[:].rearrange("(k p) g -> p k g", p=128)
        for q in range(4):
            nc.scalar.dma_start(
                out=bwT_sb[:, 2 * q:2 * q + 2, :], in_=bwT_view[:, 2 * q:2 * q + 2, :]
            )
        # bw natural [g, d] -> [128, gc, d]  (mm2 moving operand)
        bw_sb = const_pool.tile([128, KG, D], BF16)
        bw_view = bw_d[:].rearrange("(gc p) d -> p gc d", p=128)
        for q in range(4):
            nc.scalar.dma_start(
                out=bw_sb[:, q:q + 1, :], in_=bw_view[:, q:q + 1, :]
            )

        # ones column: moving operand of the row-sum matmuls (and warm-up
        # stationary)
        ones_c = const_pool.tile([128, 1], BF16)
        nc.vector.memset(ones_c, 1.0)

        # HAM warm-up: the PE boots throttled to 1.2 GHz and only reaches
        # 2.4 GHz after ~3.4us of sustained activity. Burn that window on
        # dummy matmuls while the first qzT/bwT chunks are still in flight.
        warm_mv = const_pool.tile([128, MEGA], BF16)
        nc.vector.memset(warm_mv, 0.0)
        warm_ps = qg_psum.tile([1, MEGA], F32, name="warm_ps", tag="qg_ps")
        for w in range(N_WARM):
            nc.tensor.matmul(
                warm_ps, ones_c, warm_mv,
                start=(w == 0), stop=(w == N_WARM - 1),
            )

        def load_qzT(t):
            # fill megas load in small chunks (earlier first matmul);
            # steady state uses 512KB halves (better DMA efficiency)
            qzT = in_pool.tile([128, KD, MEGA], BF16, name="qzT")
            qzT_view = qzT_d[:, t * MEGA:(t + 1) * MEGA].rearrange(
                "(k p) r -> p k r", p=128
            )
            nq = 8 if t == 0 else (4 if t < 3 else 2)
            step = KD // nq
            for q in range(nq):
                nc.sync.dma_start(
                    out=qzT[:, step * q:step * (q + 1), :],
                    in_=qzT_view[:, step * q:step * (q + 1), :],
                )
            return qzT

        # Load issues are software-pipelined two megas ahead; the sync ring
        # carries nothing but these loads so they are never head-of-line
        # blocked by a store's semaphore wait.
        pend_qzT = [load_qzT(0), load_qzT(1)]

        for t in range(NMEGA):
            qzT = pend_qzT.pop(0)
            if t + 2 < NMEGA:
                pend_qzT.append(load_qzT(t + 2))

            # ---- mm1: qgT[gc] = sum_k bwT[:,k,gc].T @ qzT[:,k,:]  -> relu ----
            qgr = qgr_pool.tile([128, KG, MEGA], BF16, name="qgr")
            rsc_ps = None
            for gc in range(KG):
                qg_ps = qg_psum.tile([128, MEGA], F32, name="qg_ps")
                for k in range(KD):
                    nc.tensor.matmul(
                        qg_ps,
                        bwT_sb[:, k, gc * 128:(gc + 1) * 128],
                        qzT[:, k, :],
                        start=(k == 0),
                        stop=(k == KD - 1),
                    )
                nc.scalar.activation(qgr[:, gc, :], qg_ps, AF.Relu)
                if gc == 1:
                    # allocated after gc0/gc1's psum tiles so the pool
                    # rotation never makes an mm1 group wait on the
                    # still-live rowsum column tile
                    rsc_ps = rs_psum.tile([128, NSUB], F32, name="rsc_ps", tag="qg_ps")

            # ---- mm2: msg[s] = sum_gc qgr[:,gc,s].T @ bw[gc], scaled ----
            # Row-sum columns ride between the two halves: for each s, four
            # 1-column matmuls (same stationary slices as mm2, so their relu
            # dependencies are already satisfied) put sum_g qg[g, p] into
            # rsc_ps[:, s]; max+reciprocal on DVE overlap the h=1 half.
            msg_sb = out_pool.tile([128, NSUB, D], BF16, name="msg_sb")
            sc_sb = small_pool.tile([128, NSUB], F32, name="sc_sb")
            for s in range(NSUB):
                m_ps = []
                for h in range(2):
                    mp = msg_psum.tile([128, 512], F32, name="m_ps")
                    m_ps.append(mp)
                    for gc in range(KG):
                        nc.tensor.matmul(
                            mp,
                            qgr[:, gc, s * 128:(s + 1) * 128],
                            bw_sb[:, gc, h * 512:(h + 1) * 512],
                            start=(gc == 0),
                            stop=(gc == KG - 1),
                        )
                    if h == 0:
                        for gc in range(KG):
                            nc.tensor.matmul(
                                rsc_ps[:, s:s + 1],
                                qgr[:, gc, s * 128:(s + 1) * 128],
                                ones_c,
                                start=(gc == 0),
                                stop=(gc == KG - 1),
                                skip_group_check=True,
                            )
                        nc.vector.tensor_scalar_max(
                            sc_sb[:, s:s + 1], rsc_ps[:, s:s + 1], EPS
                        )
                        nc.vector.reciprocal(sc_sb[:, s:s + 1], sc_sb[:, s:s + 1])
                for h in range(2):
                    nc.vector.tensor_scalar_mul(
                        msg_sb[:, s, h * 512:(h + 1) * 512],
                        m_ps[h],
                        sc_sb[:, s:s + 1],
                    )
                # output stores ride the ACT ring (weights are long done);
                # the sync ring stays loads-only
                nc.scalar.dma_start(
                    out=msg_d[t * MEGA + s * 128:t * MEGA + (s + 1) * 128, :],
                    in_=msg_sb[:, s, :],
                )

    nc.compile()
    return nc


_NC_CACHE = None


def _get_nc():
    global _NC_CACHE
    if _NC_CACHE is None:
        _NC_CACHE = build_kernel()
    return _NC_CACHE


def kernel(qz: np.ndarray, binary_weight: np.ndarray) -> np.ndarray:
    qz = np.asarray(qz, dtype=np.float32)
    bw = np.ascontiguousarray(np.asarray(binary_weight, dtype=np.float32))
    assert qz.shape == (B, C, P, D), qz.shape
    assert bw.shape == (B, G, D), bw.shape

    nc = _get_nc()
    in_maps = []
    for i in range(N_CORES):
        qzT = np.ascontiguousarray(qz[i].reshape(R, D).T).astype(NP_BF16)
        bwi = bw[i].astype(NP_BF16)                              # [G, D]
        bwT = np.ascontiguousarray(bw[i].T).astype(NP_BF16)      # [D, G]
        in_maps.append({"qzT": qzT, "bw": bwi, "bwT": bwT})
    res = run_bass_kernel_spmd(nc, in_maps, core_ids=list(range(N_CORES)))
    out = np.stack(
        [
            res.results[i]["msg"].astype(np.float32).reshape(C, P, D)
            for i in range(N_CORES)
        ],
        axis=0,
    )
    return out


# revision 15
# speedup vs baseline: 1.1903x; 1.0049x over previous
"""Trainium2 Bass kernel for DynamicPTTopicModeling.

Computes, per batch b (one batch per NeuronCore, 8 cores):
    qg  = relu(qz @ bw.T)            # [R=8192, G=512], contraction over d=1024
    den = max(sum_g qg, 1e-6)        # per-row L1 norm
    msg = (qg @ bw) / den            # [R, D=1024]

Sharding: batch b across the 8 NeuronCores, fully data-parallel (one batch
per core, no collectives).

The PE contracts over the partition dim for both operands, so qz must enter
mm1 with d on partitions. kernel() transposes qz/bw on the host while
building the per-core shards (layout marshalling, same class as the
reshape/shard step) so the device runs a pure matmul stream.

All tensors move and compute in bf16 (PSUM accumulation stays fp32):
  - same PE throughput as f32r (1 cycle/row), but weight loads get FWL
    (fast weight load) instead of the ~427-cycle fp32 path, so LDWEIGHTS
    fully hides under the 512-cycle matmuls;
  - halves DMA bytes in both directions, which removes the PE starvation
    in the first ~3 mega-tiles and halves the output tail;
  - measured end-to-end relative error ~3e-3 vs the fp32 reference
    (tolerance 2e-2): the matmul chain is short and PSUM accumulates fp32.

Per-core strategy (16 "mega-tiles" of 512 rows):
  - mm1 produces qg TRANSPOSED ([g, p] layout): stationary = host-pretransposed
    bwT slices, moving = host-pretransposed qzT chunks (N=512). mm2 then
    consumes qg slices directly as its stationary with bw natural as moving.
  - Row-sums over g (the partition dim here) ride mm2: after each 128-row
    block's first mm2 group, four 1-column matmuls qgr_slice.T @ ones
    accumulate sum_g qg into a [128, 1] PSUM column — same stationary and
    same dependencies as the mm2 matmuls (so they never add a wait), and
    den lands directly in per-partition layout for the scaled drain.
    No ones-stationary row-sum passes, no PE transposes.

Schedule notes:
  - The PE clock is gated by HAM: cold = 1.2 GHz until ~3.4us of sustained
    activity. A burst of dummy warm-up matmuls runs while the first DMAs are
    in flight so the real stream starts at 2.4 GHz.
  - Trn2 has two HWDGE rings (sync + ACT), FIFO per issuing engine. Sync
    ring = qzT loads only; ACT ring = weights then msg stores. All scaled
    PSUM->SBUF multiplies run on DVE so the ACT queue is just relu +
    dma_start and a waiting store never delays compute or loads.
"""
from contextlib import ExitStack

import ml_dtypes
import numpy as np

import concourse.bass as bass
import concourse.tile as tile
from concourse import bacc, mybir
from concourse.bass_utils import run_bass_kernel_spmd

F32 = mybir.dt.float32
BF16 = mybir.dt.bfloat16
AF = mybir.ActivationFunctionType
NP_BF16 = ml_dtypes.bfloat16

B, C, P, D, G = 8, 16, 512, 1024, 512
R = C * P            # 8192 rows per batch
MEGA = 512           # rows per mega-tile
NSUB = MEGA // 128   # 4
NMEGA = R // MEGA    # 16
KD = D // 128        # 8 d-chunks
KG = G // 128        # 4 g-chunks
EPS = 1e-6
N_CORES = 8
N_WARM = 8           # dummy matmuls to flip the HAM clock gate while DMAs fly
                     # (8 x 427ns cold = the full 3.4us HAM busy window)


def build_kernel():
    nc = bacc.Bacc("TRN2", target_bir_lowering=False)
    # Inputs are host-pretransposed and host-converted to bf16.
    qzT_d = nc.dram_tensor("qzT", [D, R], BF16, kind="ExternalInput")
    bw_d = nc.dram_tensor("bw", [G, D], BF16, kind="ExternalInput")
    bwT_d = nc.dram_tensor("bwT", [D, G], BF16, kind="ExternalInput")
    msg_d = nc.dram_tensor("msg", [R, D], BF16, kind="ExternalOutput")

    with tile.TileContext(nc) as tc, ExitStack() as ctx:
        const_pool = ctx.enter_context(tc.tile_pool(name="const", bufs=1))
        in_pool = ctx.enter_context(tc.tile_pool(name="inp", bufs=3))
        qgr_pool = ctx.enter_context(tc.tile_pool(name="qgrp", bufs=2))
        out_pool = ctx.enter_context(tc.tile_pool(name="outp", bufs=2))
        small_pool = ctx.enter_context(tc.tile_pool(name="smallp", bufs=2))
        qg_psum = ctx.enter_context(tc.tile_pool(name="qgps", bufs=4, space="PSUM"))
        msg_psum = ctx.enter_context(tc.tile_pool(name="msgps", bufs=4, space="PSUM"))
        # warm/rowsum psum tiles share the qg pool's slots (tag "qg_ps")
        rs_psum = qg_psum

        # Weights go on the second HWDGE ring (nc.scalar) so they don't queue
        # behind the qzT stream; bwT first and in quarters — it gates mm1.
        # bwT [d, g] -> [128, k, G]  (mm1 stationary)
        bwT_sb = const_pool.tile([128, KD, G], BF16)
        bwT_view = bwT_d[:].rearrange("(k p) g -> p k g", p=128)
        for q in range(4):
            nc.scalar.dma_start(
                out=bwT_sb[:, 2 * q:2 * q + 2, :], in_=bwT_view[:, 2 * q:2 * q + 2, :]
            )
        # bw natural [g, d] -> [128, gc, d]  (mm2 moving operand)
        bw_sb = const_pool.tile([128, KG, D], BF16)
        bw_view = bw_d[:].rearrange("(gc p) d -> p gc d", p=128)
        for q in range(4):
            nc.scalar.dma_start(
                out=bw_sb[:, q:q + 1, :], in_=bw_view[:, q:q + 1, :]
            )

        # ones column: moving operand of the row-sum matmuls (and warm-up
        # stationary)
        ones_c = const_pool.tile([128, 1], BF16)
        nc.vector.memset(ones_c, 1.0)

        # HAM warm-up: the PE boots throttled to 1.2 GHz and only reaches
        # 2.4 GHz after ~3.4us of sustained activity. Burn that window on
        # dummy matmuls while the first qzT/bwT chunks are still in flight.
        warm_mv = const_pool.tile([128, MEGA], BF16)
        nc.vector.memset(warm_mv, 0.0)
        warm_ps = qg_psum.tile([1, MEGA], F32, name="warm_ps", tag="qg_ps")
        for w in range(N_WARM):
            nc.tensor.matmul(
                warm_ps, ones_c, warm_mv,
                start=(w == 0), stop=(w == N_WARM - 1),
            )

        def load_qzT(t):
            # fill megas load in small chunks (earlier first matmul);
            # steady state uses 512KB halves (better DMA efficiency)
            qzT = in_pool.tile([128, KD, MEGA], BF16, name="qzT")
            qzT_view = qzT_d[:, t * MEGA:(t + 1) * MEGA].rearrange(
                "(k p) r -> p k r", p=128
            )
            nq = 8 if t == 0 else (4 if t < 3 else 2)
            step = KD // nq
            for q in range(nq):
                nc.sync.dma_start(
                    out=qzT[:, step * q:step * (q + 1), :],
                    in_=qzT_view[:, step * q:step * (q + 1), :],
                )
            return qzT

        # Load issues are software-pipelined two megas ahead; the sync ring
        # carries nothing but these loads so they are never head-of-line
        # blocked by a store's semaphore wait.
        pend_qzT = [load_qzT(0), load_qzT(1)]

        for t in range(NMEGA):
            qzT = pend_qzT.pop(0)
            if t + 2 < NMEGA:
                pend_qzT.append(load_qzT(t + 2))

            # ---- mm1: qgT[gc] = sum_k bwT[:,k,gc].T @ qzT[:,k,:]  -> relu ----
            qgr = qgr_pool.tile([128, KG, MEGA], BF16, name="qgr")
            rsc_ps = None
            for gc in range(KG):
                qg_ps = qg_psum.tile([128, MEGA], F32, name="qg_ps")
                for k in range(KD):
                    nc.tensor.matmul(
                        qg_ps,
                        bwT_sb[:, k, gc * 128:(gc + 1) * 128],
                        qzT[:, k, :],
                        start=(k == 0),
                        stop=(k == KD - 1),
                    )
                nc.scalar.activation(qgr[:, gc, :], qg_ps, AF.Relu)
                if gc == 1:
                    # allocated after gc0/gc1's psum tiles so the pool
                    # rotation never makes an mm1 group wait on the
                    # still-live rowsum column tile
                    rsc_ps = rs_psum.tile([128, NSUB], F32, name="rsc_ps", tag="qg_ps")

            # ---- mm2: msg[s] = sum_gc qgr[:,gc,s].T @ bw[gc], scaled ----
            # Row-sum columns ride between the two halves: for each s, four
            # 1-column matmuls (same stationary slices as mm2, so their relu
            # dependencies are already satisfied) put sum_g qg[g, p] into
            # rsc_ps[:, s]; max+reciprocal on DVE overlap the h=1 half.
            msg_sb = out_pool.tile([128, NSUB, D], BF16, name="msg_sb")
            sc_sb = small_pool.tile([128, NSUB], F32, name="sc_sb")
            for s in range(NSUB):
                m_ps = []
                for h in range(2):
                    mp = msg_psum.tile([128, 512], F32, name="m_ps")
                    m_ps.append(mp)
                    for gc in range(KG):
                        nc.tensor.matmul(
                            mp,
                            qgr[:, gc, s * 128:(s + 1) * 128],
                            bw_sb[:, gc, h * 512:(h + 1) * 512],
                            start=(gc == 0),
                            stop=(gc == KG - 1),
                        )
                    if h == 0:
                        for gc in range(KG):
                            nc.tensor.matmul(
                                rsc_ps[:, s:s + 1],
                                qgr[:, gc, s * 128:(s + 1) * 128],
                                ones_c,
                                start=(gc == 0),
                                stop=(gc == KG - 1),
                                skip_group_check=True,
                            )
                        nc.vector.tensor_scalar_max(
                            sc_sb[:, s:s + 1], rsc_ps[:, s:s + 1], EPS
                        )
                        nc.vector.reciprocal(sc_sb[:, s:s + 1], sc_sb[:, s:s + 1])
                # output stores ride the ACT ring (weights are long done);
                # the sync ring stays loads-only. The final mega stores per
                # 512-col half so the very last DMA launches one mul earlier.
                rows = slice(t * MEGA + s * 128, t * MEGA + (s + 1) * 128)
                for h in range(2):
                    nc.vector.tensor_scalar_mul(
                        msg_sb[:, s, h * 512:(h + 1) * 512],
                        m_ps[h],
                        sc_sb[:, s:s + 1],
                    )
                    if t == NMEGA - 1 and s == NSUB - 1:
                        nc.scalar.dma_start(
                            out=msg_d[rows, h * 512:(h + 1) * 512],
                            in_=msg_sb[:, s, h * 512:(h + 1) * 512],
                        )
                if not (t == NMEGA - 1 and s == NSUB - 1):
                    nc.scalar.dma_start(
                        out=msg_d[rows, :],
                        in_=msg_sb[:, s, :],
                    )

    nc.compile()
    return nc


_NC_CACHE = None


def _get_nc():
    global _NC_CACHE
    if _NC_CACHE is None:
        _NC_CACHE = build_kernel()
    return _NC_CACHE


def kernel(qz: np.ndarray, binary_weight: np.ndarray) -> np.ndarray:
    qz = np.asarray(qz, dtype=np.float32)
    bw = np.ascontiguousarray(np.asarray(binary_weight, dtype=np.float32))
    assert qz.shape == (B, C, P, D), qz.shape
    assert bw.shape == (B, G, D), bw.shape

    nc = _get_nc()
    in_maps = []
    for i in range(N_CORES):
        qzT = np.ascontiguousarray(qz[i].reshape(R, D).T).astype(NP_BF16)
        bwi = bw[i].astype(NP_BF16)                              # [G, D]
        bwT = np.ascontiguousarray(bw[i].T).astype(NP_BF16)      # [D, G]
        in_maps.append({"qzT": qzT, "bw": bwi, "bwT": bwT})
    res = run_bass_kernel_spmd(nc, in_maps, core_ids=list(range(N_CORES)))
    out = np.stack(
        [
            res.results[i]["msg"].astype(np.float32).reshape(C, P, D)
            for i in range(N_CORES)
        ],
        axis=0,
    )
    return out
